# revision 1
# baseline (speedup 1.0000x reference)
"""Causal self-attention on 8 trn2 NeuronCores.

Sharding: core c -> (batch b = c//2, head-group g = c%2 of 8 heads).
Each core computes qkv for its (batch, head-group), causal attention for
its 8 heads, and the row-slice of the output projection for its 512
channels. Host sums the two per-batch partial projections.

Kernel design:
- x is passed transposed per batch (xT [1024, 2048]) so the contraction
  dim (model channels) lands on SBUF partitions for all qkv matmuls.
- Scores are computed transposed (S^T [keys, queries]): softmax
  denominator comes from a ones-column appended to V inside the PV
  matmul; normalization is applied to the unnormalized head outputs
  (fast reciprocal + gpsimd partition-broadcast + one multiply).
- Head-PAIR packing: the two heads of a feature tile occupy PE row
  groups 0-63 / 64-127; their K=64 score matmuls are emitted adjacently
  so the PE runs them concurrently in different row groups (~2x).
  One wide exp [128, 1024] covers both heads' score blocks.
- Causal: key-tile blocks below the diagonal run unmasked, blocks above
  are skipped, the 4 diagonal blocks per query macro get a 0/1
  multiplicative mask post-exp.
- All matmuls in float32r (single-pass reduced fp32, ~250ns/128x128x512).
"""

import sys

sys.path.insert(0, "/opt/trn_rl_repo")

import numpy as np
import ml_dtypes

import concourse.bass as bass
import concourse.mybir as mybir
import concourse.tile as tile
from concourse import bacc
from concourse.bass_utils import run_bass_kernel_spmd

# Problem shape (hardcoded per the contract).
B = 4
T = 2048
C = 1024
N_HEADS = 16
HD = 64
N_CORES = 8

# Per-core sharding.
H_PER_CORE = 8          # heads per core
CH = H_PER_CORE * HD    # 512 channels per core
KC = C // 128           # 8 contraction tiles over model dim
FT = CH * 2 // 128      # 8 feature tiles for q+k ([q0..q3, k0..k3])
TT = T // 128           # 16 token tiles
QM = T // 512           # 4 query macros
NQ = 4                  # token quarters in phase Q
SCALE = HD ** -0.5

F32 = mybir.dt.float32
F32R = mybir.dt.float32r
BF16 = mybir.dt.bfloat16

_CACHE = {}


def build_kernel(debug=False):
    nc = bacc.Bacc(target_bir_lowering=False)

    xT = nc.dram_tensor("xT", [C, T], F32R, kind="ExternalInput")
    w_qk = nc.dram_tensor("w_qk", [FT, 128, KC, 128], F32R, kind="ExternalInput")
    w_v = nc.dram_tensor("w_v", [KC, 128, CH], F32R, kind="ExternalInput")
    w_pj = nc.dram_tensor("w_pj", [CH // 128, 128, C], F32R, kind="ExternalInput")
    masks = nc.dram_tensor("masks", [128, 4, 1024], F32R, kind="ExternalInput")
    ones_d = nc.dram_tensor("ones_d", [128, 64], F32R, kind="ExternalInput")
    y = nc.dram_tensor("y", [T, C], F32, kind="ExternalOutput")
    if debug:
        dbg_qk = nc.dram_tensor("dbg_qk", [FT, 128, T], F32R, kind="ExternalOutput")
        dbg_v = nc.dram_tensor("dbg_v", [TT, 128, H_PER_CORE, HD + 1], F32R,
                               kind="ExternalOutput")
        dbg_o = nc.dram_tensor("dbg_o", [CH // 128, 128, T], F32R,
                               kind="ExternalOutput")

    with tile.TileContext(nc) as tc:
        with tc.tile_pool(name="big", bufs=1) as big:
            # ---- resident SBUF tensors ----
            qkT = [big.tile([128, T], F32R, tag=f"qkT{ft}", name=f"qkT{ft}")
                   for ft in range(FT)]
            vt = [big.tile([128, H_PER_CORE, HD + 1], F32R, tag=f"v{tt}",
                           name=f"v{tt}") for tt in range(TT)]
            outT = [big.tile([128, T], F32R, tag=f"outT{ct}", name=f"outT{ct}")
                    for ct in range(CH // 128)]
            ones_sb = big.tile([1, HD], F32R, tag="ones")

            nc.sync.dma_start(out=ones_sb, in_=ones_d[0:1, 0:HD])
            ones_col = ones_d[:, 0:H_PER_CORE].rearrange("p (a b) -> p a b", b=1)
            for tt in range(TT):
                nc.sync.dma_start(out=vt[tt][:, :, HD:HD + 1], in_=ones_col)

            # ---- phase Q: qkv projections, one token quarter at a time ----
            with (
                tc.tile_pool(name="xtp", bufs=2) as xtp,
                tc.tile_pool(name="wqs", bufs=2) as wqs,
                tc.tile_pool(name="wvs", bufs=3) as wvs,
                tc.tile_pool(name="psq", bufs=3, space="PSUM") as psq,
                tc.tile_pool(name="psv", bufs=1, space="PSUM") as psv,
            ):
                for tq in range(NQ):
                    xts = []
                    for kc in range(KC):
                        xt = xtp.tile([128, 512], F32R, tag=f"xt{kc}",
                                      name=f"xt{kc}", bufs=2)
                        nc.sync.dma_start(
                            out=xt, in_=xT[kc * 128:(kc + 1) * 128,
                                           tq * 512:(tq + 1) * 512])
                        xts.append(xt)

                    # q^T / k^T: [feat, tok] tiles
                    for ft in range(FT):
                        wq8 = wqs.tile([128, KC, 128], F32R, tag="wq")
                        nc.sync.dma_start(out=wq8, in_=w_qk[ft])
                        acc = psq.tile([128, 512], F32, tag="qk")
                        for kc in range(KC):
                            nc.tensor.matmul(
                                acc[:], wq8[:, kc, :], xts[kc][:],
                                start=(kc == 0), stop=(kc == KC - 1))
                        nc.scalar.copy(
                            qkT[ft][:, tq * 512:(tq + 1) * 512], acc[:])

                    # v: [tok, feat] tiles + ones col; kc-outer so w_v
                    # streams once per quarter, 4 token-tile psums live
                    vaccs = [psv.tile([128, CH], F32, tag=f"v{i}", name=f"vac{i}")
                             for i in range(4)]
                    for kc in range(KC):
                        wv = wvs.tile([128, CH], F32R, tag="wv")
                        nc.sync.dma_start(out=wv, in_=w_v[kc])
                        for i in range(4):
                            tt = tq * 4 + i
                            nc.tensor.matmul(
                                vaccs[i][:],
                                xts[kc][:, i * 128:(i + 1) * 128],
                                wv[:],
                                start=(kc == 0), stop=(kc == KC - 1))
                    for i in range(4):
                        tt = tq * 4 + i
                        nc.scalar.copy(
                            vt[tt][:, :, 0:HD],
                            vaccs[i][:].rearrange("p (h d) -> p h d",
                                                  h=H_PER_CORE))

            # ---- phase A: causal attention, head pairs packed ----
            with (
                tc.tile_pool(name="pts", bufs=4) as pts,
                tc.tile_pool(name="sml", bufs=4) as sml,
                tc.tile_pool(name="msk", bufs=1) as mskp,
                tc.tile_pool(name="pssw", bufs=2, space="PSUM") as pssw,
                tc.tile_pool(name="pso", bufs=2, space="PSUM") as pso,
            ):
                mask_sb = mskp.tile([128, 4, 1024], F32R, tag="masks")
                nc.sync.dma_start(out=mask_sb, in_=masks[:])
                for p in range(4):          # head pair = heads 2p, 2p+1
                    qTh = qkT[p]
                    kTh = qkT[4 + p]
                    for qm in range(QM):
                        nkt = 4 * qm + 4
                        oacc = [pso.tile([HD + 1, 512], F32, tag=f"o{hh}",
                                         name=f"o{hh}") for hh in range(2)]
                        for kt in range(nkt):
                            j = kt - 4 * qm     # >=0 on diagonal blocks
                            o0 = max(j, 0) * 128   # first valid query col
                            sw = pssw.tile([128, 1024], F32, tag="sw")
                            for hh in range(2):
                                nc.tensor.matmul(
                                    sw[:, hh * 512 + o0:(hh + 1) * 512],
                                    kTh[hh * 64:(hh + 1) * 64,
                                        kt * 128:(kt + 1) * 128],
                                    qTh[hh * 64:(hh + 1) * 64,
                                        qm * 512 + o0:(qm + 1) * 512],
                                    start=True, stop=True)
                            pt = pts.tile([128, 1024], F32R, tag="pT")
                            swv = sw[:].rearrange("p (a q) -> p a q", a=2)
                            ptv = pt[:].rearrange("p (a q) -> p a q", a=2)
                            nc.scalar.activation(
                                ptv[:, :, o0:512], swv[:, :, o0:512],
                                mybir.ActivationFunctionType.Exp, scale=SCALE)
                            if j >= 0:      # diagonal block: 0/1 mask both heads
                                mv = mask_sb[:, j, :].rearrange(
                                    "p (a q) -> p a q", a=2)
                                nc.vector.tensor_mul(
                                    ptv[:, :, o0:512], ptv[:, :, o0:512],
                                    mv[:, :, o0:512])
                            for hh in range(2):
                                h = 2 * p + hh
                                nc.tensor.matmul(
                                    oacc[hh][:, o0:512],
                                    vt[kt][:, h, :],
                                    pt[:, hh * 512 + o0:(hh + 1) * 512],
                                    start=(kt == 0), stop=(kt == nkt - 1),
                                    skip_group_check=True)
                        for hh in range(2):
                            den = sml.tile([1, 512], F32, tag="den")
                            nc.vector.tensor_copy(den[:], oacc[hh][HD:HD + 1, :])
                            rd = sml.tile([1, 512], F32, tag="rd")
                            nc.vector.reciprocal_approx_fast(rd[:], den[:])
                            bcs = sml.tile([HD, 512], F32, tag="bcs")
                            nc.gpsimd.partition_broadcast(bcs[:], rd[:])
                            nc.vector.tensor_mul(
                                outT[p][hh * 64:(hh + 1) * 64,
                                        qm * 512:(qm + 1) * 512],
                                oacc[hh][0:HD, :], bcs[:])

            if debug:
                for ft in range(FT):
                    nc.sync.dma_start(out=dbg_qk[ft], in_=qkT[ft][:])
                for tt in range(TT):
                    nc.sync.dma_start(out=dbg_v[tt], in_=vt[tt][:])
                for ct in range(CH // 128):
                    nc.sync.dma_start(out=dbg_o[ct], in_=outT[ct][:])

            # ---- phase P: output projection (row-parallel slice) ----
            with (
                tc.tile_pool(name="wps", bufs=2) as wps,
                tc.tile_pool(name="ysb", bufs=3) as ysbp,
                tc.tile_pool(name="psp", bufs=3, space="PSUM") as psp,
            ):
                for nf in range(2):
                    wpj = []
                    for ct in range(CH // 128):
                        w = wps.tile([128, 512], F32R, tag=f"wpj{ct}",
                                     name=f"wpj{ct}")
                        nc.sync.dma_start(
                            out=w, in_=w_pj[ct][:, nf * 512:(nf + 1) * 512])
                        wpj.append(w)
                    for tt in range(TT):
                        accp = psp.tile([128, 512], F32, tag="pp")
                        for ct in range(CH // 128):
                            nc.tensor.matmul(
                                accp[:],
                                outT[ct][:, tt * 128:(tt + 1) * 128],
                                wpj[ct][:],
                                start=(ct == 0), stop=(ct == CH // 128 - 1))
                        ys = ysbp.tile([128, 512], F32, tag="ys")
                        nc.scalar.copy(ys[:], accp[:])
                        nc.sync.dma_start(
                            out=y[tt * 128:(tt + 1) * 128,
                                  nf * 512:(nf + 1) * 512],
                            in_=ys[:])

    nc.compile()
    return nc


def _make_masks():
    k = np.arange(128)[:, None, None]
    j = np.arange(4)[None, :, None]
    q = np.arange(512)[None, None, :]
    m = (j * 128 + k <= q)                       # [128, 4, 512]
    m2 = np.concatenate([m, m], axis=2)          # [128, 4, 1024] (both heads)
    return m2.astype(np.float32)


def make_in_maps(x, w_qkv, w_proj):
    masks = _make_masks()
    ones = np.ones((128, 64), dtype=np.float32)
    in_maps = []
    for c in range(N_CORES):
        b, g = c // 2, c % 2
        xTv = np.ascontiguousarray(x[b].T)
        wq = w_qkv[:, g * CH:(g + 1) * CH]
        wk = w_qkv[:, C + g * CH:C + (g + 1) * CH]
        stacked = np.concatenate([wq, wk], axis=1)           # [1024, 1024]
        # [ft, c_within_tile, kc, f]: tile ft, contraction row c of chunk
        # kc, feature f -> stacked[kc*128 + c, ft*128 + f]
        w_qk = np.ascontiguousarray(
            stacked.reshape(KC, 128, FT, 128).transpose(2, 1, 0, 3))
        w_v = np.ascontiguousarray(
            w_qkv[:, 2 * C + g * CH:2 * C + (g + 1) * CH].reshape(KC, 128, CH))
        w_pj = np.ascontiguousarray(
            w_proj[g * CH:(g + 1) * CH, :].reshape(CH // 128, 128, C))
        in_maps.append({
            "xT": xTv, "w_qk": w_qk, "w_v": w_v, "w_pj": w_pj,
            "masks": masks, "ones_d": ones,
        })
    return in_maps


def kernel(x, w_qkv, w_proj):
    x = np.asarray(x, dtype=np.float32)
    w_qkv = np.asarray(w_qkv, dtype=np.float32)
    w_proj = np.asarray(w_proj, dtype=np.float32)

    if "nc" not in _CACHE:
        _CACHE["nc"] = build_kernel()
    nc = _CACHE["nc"]

    in_maps = make_in_maps(x, w_qkv, w_proj)
    res = run_bass_kernel_spmd(nc, in_maps, core_ids=list(range(N_CORES)))
    _CACHE["last_result"] = res

    y = np.empty((B, T, C), dtype=np.float32)
    for b in range(B):
        y[b] = res.results[2 * b]["y"] + res.results[2 * b + 1]["y"]
    return y



# revision 4
# speedup vs baseline: 1.2642x; 1.2642x over previous
"""Causal self-attention on 8 trn2 NeuronCores (bf16 rewrite).

Sharding: core c -> (batch b = c//2, head-group g = c%2 of 8 heads).
Each core computes qkv for its (batch, head-group), causal attention for
its 8 heads, and the row-slice of the output projection for its 512
channels. Host sums the two per-batch partial projections.

V1 changes vs fp32r baseline:
- all matmul operands bf16 (FWL ldweights, no narrow-N fp32r penalty,
  half DMA), PSUM stays fp32.
- weights DMA'd once, resident in SBUF.
- PSUM->SBUF copies on DVE (vector), not ScalarE; ACT does only exp.
- per-quarter software pipeline: Q(tq) qkv -> A(qm=tq) attention ->
  P(tq) projection, so exp (ACT-bound) overlaps PE work.
- static 8-bank PSUM aliasing: sw0/sw1 [128,1024] (scores ping-pong,
  qk-proj chains, proj chains), vo0-3 [128,512] (v-acc + attention
  out accumulators).
"""

import sys

sys.path.insert(0, "/opt/trn_rl_repo")

import numpy as np
import ml_dtypes

import concourse.bass as bass
import concourse.mybir as mybir
import concourse.tile as tile
from concourse import bacc
from concourse.bass_utils import run_bass_kernel_spmd

B = 4
T = 2048
C = 1024
N_HEADS = 16
HD = 64
N_CORES = 8

H_PER_CORE = 8          # heads per core
CH = H_PER_CORE * HD    # 512 channels per core
KC = C // 128           # 8 contraction tiles over model dim
FT = CH * 2 // 128      # 8 feature tiles for q+k ([q0..q3, k0..k3])
TT = T // 128           # 16 token tiles
QM = T // 512           # 4 query macros (= quarters)
NQ = 4
CT = CH // 128          # 4 outT channel tiles
SCALE = HD ** -0.5

F32 = mybir.dt.float32
BF16 = mybir.dt.bfloat16

_CACHE = {}


def build_kernel():
    nc = bacc.Bacc(target_bir_lowering=False)

    xT = nc.dram_tensor("xT", [C, T], BF16, kind="ExternalInput")
    w_qk = nc.dram_tensor("w_qk", [FT, 128, KC, 128], BF16, kind="ExternalInput")
    w_v = nc.dram_tensor("w_v", [KC, 128, CH], BF16, kind="ExternalInput")
    w_pj = nc.dram_tensor("w_pj", [CT, 128, C], BF16, kind="ExternalInput")
    masks = nc.dram_tensor("masks", [128, 4, 1024], BF16, kind="ExternalInput")
    ones_d = nc.dram_tensor("ones_d", [128, 64], BF16, kind="ExternalInput")
    y = nc.dram_tensor("y", [T, C], F32, kind="ExternalOutput")

    with tile.TileContext(nc) as tc:
        with (
            tc.tile_pool(name="big", bufs=1) as big,
            tc.tile_pool(name="xtp", bufs=2) as xtp,
            tc.tile_pool(name="pts", bufs=4) as pts,
            tc.tile_pool(name="sml", bufs=4) as sml,
            tc.tile_pool(name="ysb", bufs=3) as ysbp,
            tc.tile_pool(name="ps", bufs=1, space="PSUM") as ps,
        ):
            # ---- resident SBUF tensors ----
            qkT = [big.tile([128, T], BF16, tag=f"qkT{ft}", name=f"qkT{ft}") for ft in range(FT)]
            vt = [big.tile([128, H_PER_CORE, HD + 1], BF16, tag=f"v{tt}", name=f"v{tt}")
                  for tt in range(TT)]
            outT = [big.tile([128, T], BF16, tag=f"outT{ct}", name=f"outT{ct}")
                    for ct in range(CT)]
            wq = [big.tile([128, KC, 128], BF16, tag=f"wq{ft}", name=f"wq{ft}")
                  for ft in range(FT)]
            wv = [big.tile([128, CH], BF16, tag=f"wv{kc}", name=f"wv{kc}") for kc in range(KC)]
            wpj = [big.tile([128, C], BF16, tag=f"wpj{ct}", name=f"wpj{ct}") for ct in range(CT)]
            mask_sb = big.tile([128, 4, 1024], BF16, tag="masks", name="masks")

            # PSUM: exactly 8 banks, statically aliased across phases.
            sw01 = [ps.tile([128, 1024], F32, tag=f"sw{i}", name=f"sw{i}") for i in range(2)]
            vo = [ps.tile([128, 512], F32, tag=f"vo{i}", name=f"vo{i}") for i in range(4)]

            # weight / mask loads (once)
            for ft in range(FT):
                nc.sync.dma_start(out=wq[ft], in_=w_qk[ft])
            for kc in range(KC):
                nc.sync.dma_start(out=wv[kc], in_=w_v[kc])
            for ct in range(CT):
                nc.sync.dma_start(out=wpj[ct], in_=w_pj[ct])
            nc.sync.dma_start(out=mask_sb, in_=masks[:])
            ones_col = ones_d[:, 0:H_PER_CORE].rearrange("p (a b) -> p a b", b=1)
            for tt in range(TT):
                nc.sync.dma_start(out=vt[tt][:, :, HD:HD + 1], in_=ones_col)

            for tq in range(NQ):
                # ---- phase Q(tq): qkv projections for token quarter tq ----
                xts = []
                for kc in range(KC):
                    xt = xtp.tile([128, 512], BF16, tag=f"xt{kc}", name=f"xt{kc}")
                    nc.sync.dma_start(
                        out=xt, in_=xT[kc * 128:(kc + 1) * 128,
                                       tq * 512:(tq + 1) * 512])
                    xts.append(xt)

                # q^T / k^T: [feat, tok] tiles; chains ping-pong through
                # sw0/sw1 half-tiles (4 slots, 2-deep WAR pipeline).
                for ft in range(FT):
                    slot = sw01[(ft // 2) % 2]
                    half = (ft % 2) * 512
                    acc = slot[:, half:half + 512]
                    for kc in range(KC):
                        nc.tensor.matmul(
                            acc, wq[ft][:, kc, :], xts[kc][:],
                            start=(kc == 0), stop=(kc == KC - 1))
                    nc.vector.tensor_copy(
                        qkT[ft][:, tq * 512:(tq + 1) * 512], acc)

                # v: [tok, feat] tiles, tt-sequential chains into vo
                for i in range(4):
                    tt = tq * 4 + i
                    vacc = vo[i]
                    for kc in range(KC):
                        nc.tensor.matmul(
                            vacc[:],
                            xts[kc][:, i * 128:(i + 1) * 128],
                            wv[kc][:],
                            start=(kc == 0), stop=(kc == KC - 1))
                    nc.vector.tensor_copy(
                        vt[tt][:, :, 0:HD],
                        vacc[:].rearrange("p (h d) -> p h d", h=H_PER_CORE))

                # ---- phase A(qm=tq): causal attention, head pairs ----
                qm = tq
                nkt = 4 * qm + 4
                for p in range(4):          # head pair = heads 2p, 2p+1
                    qTh = qkT[p]
                    kTh = qkT[4 + p]
                    ob = vo[2 * (p % 2)], vo[2 * (p % 2) + 1]
                    for kt in range(nkt):
                        j = kt - 4 * qm     # >=0 on diagonal blocks
                        o0 = max(j, 0) * 128
                        sw = sw01[kt % 2]
                        for hh in range(2):
                            nc.tensor.matmul(
                                sw[:, hh * 512 + o0:(hh + 1) * 512],
                                kTh[hh * 64:(hh + 1) * 64,
                                    kt * 128:(kt + 1) * 128],
                                qTh[hh * 64:(hh + 1) * 64,
                                    qm * 512 + o0:(qm + 1) * 512],
                                start=True, stop=True)
                        pt = pts.tile([128, 1024], BF16, tag="pT")
                        swv = sw[:].rearrange("p (a q) -> p a q", a=2)
                        ptv = pt[:].rearrange("p (a q) -> p a q", a=2)
                        nc.scalar.activation(
                            ptv[:, :, o0:512], swv[:, :, o0:512],
                            mybir.ActivationFunctionType.Exp, scale=SCALE)
                        if j >= 0:      # diagonal block: 0/1 mask both heads
                            mv = mask_sb[:, j, :].rearrange(
                                "p (a q) -> p a q", a=2)
                            nc.vector.tensor_mul(
                                ptv[:, :, o0:512], ptv[:, :, o0:512],
                                mv[:, :, o0:512])
                        for hh in range(2):
                            h = 2 * p + hh
                            nc.tensor.matmul(
                                ob[hh][0:HD + 1, o0:512],
                                vt[kt][:, h, :],
                                pt[:, hh * 512 + o0:(hh + 1) * 512],
                                start=(kt == 0), stop=(kt == nkt - 1),
                                skip_group_check=True)
                    for hh in range(2):
                        oacc = ob[hh]
                        den = sml.tile([1, 512], F32, tag="den")
                        nc.vector.tensor_copy(den[:], oacc[HD:HD + 1, 0:512])
                        rd = sml.tile([1, 512], F32, tag="rd")
                        nc.vector.reciprocal_approx_fast(rd[:], den[:])
                        bcs = sml.tile([HD, 512], F32, tag="bcs")
                        nc.gpsimd.partition_broadcast(bcs[:], rd[:])
                        nc.vector.tensor_mul(
                            outT[p][hh * 64:(hh + 1) * 64,
                                    qm * 512:(qm + 1) * 512],
                            oacc[0:HD, 0:512], bcs[:])

                # ---- phase P(tq): output projection for quarter rows ----
                for i in range(4):
                    tt = tq * 4 + i
                    for nf in range(2):
                        slot = sw01[(2 * i + nf) % 2]
                        half = ((2 * i + nf) // 2 % 2) * 512
                        accp = slot[:, half:half + 512]
                        for ct in range(CT):
                            nc.tensor.matmul(
                                accp,
                                outT[ct][:, tt * 128:(tt + 1) * 128],
                                wpj[ct][:, nf * 512:(nf + 1) * 512],
                                start=(ct == 0), stop=(ct == CT - 1))
                        ys = ysbp.tile([128, 512], F32, tag="ys")
                        nc.vector.tensor_copy(ys[:], accp)
                        nc.sync.dma_start(
                            out=y[tt * 128:(tt + 1) * 128,
                                  nf * 512:(nf + 1) * 512],
                            in_=ys[:])

    nc.compile()
    return nc


def _make_masks():
    k = np.arange(128)[:, None, None]
    j = np.arange(4)[None, :, None]
    q = np.arange(512)[None, None, :]
    m = (j * 128 + k <= q)                       # [128, 4, 512]
    m2 = np.concatenate([m, m], axis=2)          # [128, 4, 1024] (both heads)
    return m2.astype(ml_dtypes.bfloat16)


def make_in_maps(x, w_qkv, w_proj):
    bf = ml_dtypes.bfloat16
    masks = _make_masks()
    ones = np.ones((128, 64), dtype=bf)
    in_maps = []
    for c in range(N_CORES):
        b, g = c // 2, c % 2
        xTv = np.ascontiguousarray(x[b].T.astype(bf))
        wq_ = w_qkv[:, g * CH:(g + 1) * CH]
        wk_ = w_qkv[:, C + g * CH:C + (g + 1) * CH]
        stacked = np.concatenate([wq_, wk_], axis=1)         # [1024, 1024]
        w_qk = np.ascontiguousarray(
            stacked.reshape(KC, 128, FT, 128).transpose(2, 1, 0, 3).astype(bf))
        w_v = np.ascontiguousarray(
            w_qkv[:, 2 * C + g * CH:2 * C + (g + 1) * CH]
            .reshape(KC, 128, CH).astype(bf))
        w_pj = np.ascontiguousarray(
            w_proj[g * CH:(g + 1) * CH, :].reshape(CT, 128, C).astype(bf))
        in_maps.append({
            "xT": xTv, "w_qk": w_qk, "w_v": w_v, "w_pj": w_pj,
            "masks": masks, "ones_d": ones,
        })
    return in_maps


def kernel(x, w_qkv, w_proj):
    x = np.asarray(x, dtype=np.float32)
    w_qkv = np.asarray(w_qkv, dtype=np.float32)
    w_proj = np.asarray(w_proj, dtype=np.float32)

    if "nc" not in _CACHE:
        _CACHE["nc"] = build_kernel()
    nc = _CACHE["nc"]

    in_maps = make_in_maps(x, w_qkv, w_proj)
    res = run_bass_kernel_spmd(nc, in_maps, core_ids=list(range(N_CORES)))
    _CACHE["last_result"] = res

    yout = np.empty((B, T, C), dtype=np.float32)
    for b in range(B):
        yout[b] = res.results[2 * b]["y"] + res.results[2 * b + 1]["y"]
    return yout


# revision 7
# speedup vs baseline: 1.4621x; 1.1565x over previous
"""Causal self-attention on 8 trn2 NeuronCores (bf16, interleaved pipeline).

Sharding: core c -> (batch b = c//2, head-group g = c%2 of 8 heads).
Each core computes qkv for its (batch, head-group), causal attention for
its 8 heads, and the row-slice of the output projection for its 512
channels. Host sums the two per-batch partial projections.

Design:
- all matmul operands bf16 (FWL ldweights, no narrow-N fp32r penalty,
  half DMA), PSUM accumulation fp32.
- weights DMA'd once, resident in SBUF; x streamed per token quarter.
- ScalarE does only exp; PSUM->SBUF drains on DVE.
- The attention phase A(qm) is ACT(exp)-bound: PE idles ~35% inside it.
  Those idle slots are filled by interleaving the NEXT quarter's qkv
  chains (and, during A(3), the deferred projection chains) one matmul
  at a time between score/PV blocks — a software-pipelined filler queue.
- PSUM banks (8): sw0/sw1 [128,1024] score ping-pong (4), oa0/oa1
  [65,512] attention-out accumulators (2), cs0/cs1 [128,512] filler
  chain slots for qkv/proj accumulation (2).
"""

import sys

sys.path.insert(0, "/opt/trn_rl_repo")

from collections import deque

import numpy as np
import ml_dtypes

import concourse.bass as bass
import concourse.mybir as mybir
import concourse.tile as tile
from concourse import bacc
from concourse.bass_utils import run_bass_kernel_spmd

B = 4
T = 2048
C = 1024
N_HEADS = 16
HD = 64
N_CORES = 8

H_PER_CORE = 8          # heads per core
CH = H_PER_CORE * HD    # 512 channels per core
KC = C // 128           # 8 contraction tiles over model dim
FT = CH * 2 // 128      # 8 feature tiles for q+k ([q0..q3, k0..k3])
TT = T // 128           # 16 token tiles
QM = T // 512           # 4 query macros (= quarters)
NQ = 4
CT = CH // 128          # 4 outT channel tiles
SCALE = HD ** -0.5

F32 = mybir.dt.float32
BF16 = mybir.dt.bfloat16

_CACHE = {}


def build_kernel():
    nc = bacc.Bacc(target_bir_lowering=False)

    xT = nc.dram_tensor("xT", [128, KC, T], BF16, kind="ExternalInput")
    w_qk = nc.dram_tensor("w_qk", [128, FT, KC, 128], BF16,
                          kind="ExternalInput")
    w_v = nc.dram_tensor("w_v", [128, KC, CH], BF16, kind="ExternalInput")
    w_pj = nc.dram_tensor("w_pj", [128, CT, C], BF16, kind="ExternalInput")
    masks = nc.dram_tensor("masks", [128, 4, 1024], BF16, kind="ExternalInput")
    y = nc.dram_tensor("y", [T, C], F32, kind="ExternalOutput")

    with tile.TileContext(nc) as tc:
        with (
            tc.tile_pool(name="big", bufs=1) as big,
            tc.tile_pool(name="xtp", bufs=2) as xtp,
            tc.tile_pool(name="pts", bufs=4) as pts,
            tc.tile_pool(name="sml", bufs=8) as sml,
            tc.tile_pool(name="ysb", bufs=4) as ysbp,
            tc.tile_pool(name="ps", bufs=1, space="PSUM") as ps,
        ):
            # ---- resident SBUF tensors ----
            qkT = [big.tile([128, T], BF16, tag=f"qkT{ft}", name=f"qkT{ft}")
                   for ft in range(FT)]
            vt_all = big.tile([128, TT, H_PER_CORE, HD + 1], BF16,
                              tag="vt_all", name="vt_all")
            vt = [vt_all[:, tt] for tt in range(TT)]
            outT = [big.tile([128, T], BF16, tag=f"outT{ct}", name=f"outT{ct}")
                    for ct in range(CT)]
            wq_all = big.tile([128, FT, KC, 128], BF16, tag="wq_all",
                              name="wq_all")
            wq = [wq_all[:, ft] for ft in range(FT)]
            wv_all = big.tile([128, KC, CH], BF16, tag="wv_all",
                              name="wv_all")
            wv = [wv_all[:, kc] for kc in range(KC)]
            wpj_all = big.tile([128, CT, C], BF16, tag="wpj_all",
                               name="wpj_all")
            wpj = [wpj_all[:, ct] for ct in range(CT)]
            mask_sb = big.tile([128, 4, 1024], BF16, tag="masks", name="masks")

            # PSUM: 8 banks, statically assigned.
            sw01 = [ps.tile([128, 1024], F32, tag=f"sw{i}", name=f"sw{i}")
                    for i in range(2)]
            oa = [ps.tile([HD + 1, 512], F32, tag=f"oa{i}", name=f"oa{i}")
                  for i in range(2)]
            cs = [ps.tile([128, 512], F32, tag=f"cs{i}", name=f"cs{i}")
                  for i in range(2)]

            # ---- input DMAs: x quarter 0 first (unblocks Q(0)), then
            # weights in first-use order; wpj (needed last) at the end.
            xts_all = {}

            def load_x(tq):
                xt = xtp.tile([128, KC, 512], BF16, tag="xt", name="xt")
                nc.sync.dma_start(
                    out=xt, in_=xT[:, :, tq * 512:(tq + 1) * 512])
                xts_all[tq] = [xt[:, kc] for kc in range(KC)]

            nc.sync.dma_start(out=mask_sb, in_=masks[:])
            load_x(0)
            nc.sync.dma_start(out=wq_all, in_=w_qk[:])
            nc.sync.dma_start(out=wv_all, in_=w_v[:])
            nc.gpsimd.memset(vt_all[:, :, :, HD:HD + 1], 1.0)
            nc.sync.dma_start(out=wpj_all, in_=w_pj[:])

            # PE warmup burst on the mask tile: keeps the HAM busy-window
            # alive through the input DMA latency so Q(0) runs at 2.4 GHz.
            for wu in range(12):
                nc.tensor.matmul(
                    cs[wu % 2][:], mask_sb[:, 1, 0:128],
                    mask_sb[:, 2, 0:512], start=True, stop=True)

            # ---- filler machinery: queued single-matmul steps ----
            filler = deque()
            slot_ctr = [0]

            def next_slot():
                s = cs[slot_ctr[0] % 2]
                slot_ctr[0] += 1
                return s

            def queue_qk_chain(tq, ft):
                acc = next_slot()

                def mk(kc):
                    def f():
                        nc.tensor.matmul(
                            acc[:], wq[ft][:, kc, :], xts_all[tq][kc][:],
                            start=(kc == 0), stop=(kc == KC - 1))
                        if kc == KC - 1:
                            nc.vector.tensor_copy(
                                qkT[ft][:, tq * 512:(tq + 1) * 512], acc[:])
                    return f

                for kc in range(KC):
                    filler.append(mk(kc))

            def queue_v_chain(tq, i):
                acc = next_slot()
                tt = tq * 4 + i

                def mk(kc):
                    def f():
                        nc.tensor.matmul(
                            acc[:],
                            xts_all[tq][kc][:, i * 128:(i + 1) * 128],
                            wv[kc][:],
                            start=(kc == 0), stop=(kc == KC - 1))
                        if kc == KC - 1:
                            nc.vector.tensor_copy(
                                vt[tt][:, :, 0:HD],
                                acc[:].rearrange("p (h d) -> p h d",
                                                 h=H_PER_CORE))
                    return f

                for kc in range(KC):
                    filler.append(mk(kc))

            def queue_q_phase(tq):
                load_x(tq)
                for ft in range(FT):
                    queue_qk_chain(tq, ft)
                for i in range(4):
                    queue_v_chain(tq, i)

            def queue_p_chain(tq, i, nf):
                acc = next_slot()
                tt = tq * 4 + i

                def mk(ct):
                    def f():
                        nc.tensor.matmul(
                            acc[:],
                            outT[ct][:, tt * 128:(tt + 1) * 128],
                            wpj[ct][:, nf * 512:(nf + 1) * 512],
                            start=(ct == 0), stop=(ct == CT - 1))
                        if ct == CT - 1:
                            ys = ysbp.tile([128, 512], F32, tag="ys",
                                           name="ys")
                            nc.vector.tensor_copy(ys[:], acc[:])
                            nc.sync.dma_start(
                                out=y[tt * 128:(tt + 1) * 128,
                                      nf * 512:(nf + 1) * 512],
                                in_=ys[:])
                    return f

                for ct in range(CT):
                    filler.append(mk(ct))

            def queue_p_phase(tq):
                for i in range(4):
                    for nf in range(2):
                        queue_p_chain(tq, i, nf)

            def emit_filler(n):
                for _ in range(n):
                    if filler:
                        filler.popleft()()

            def drain_filler():
                while filler:
                    filler.popleft()()

            # ---- Q(0): no attention to hide it under; emit directly ----
            queue_q_phase(0)
            drain_filler()

            # ---- main loop: A(tq) with fillers from Q(tq+1) / P(<3) ----
            for tq in range(NQ):
                qm = tq
                nkt = 4 * qm + 4
                if tq < 3:
                    queue_q_phase(tq + 1)
                else:
                    for ptq in range(3):
                        queue_p_phase(ptq)
                for p in range(4):          # head pair = heads 2p, 2p+1
                    qTh = qkT[p]
                    kTh = qkT[4 + p]
                    for kt in range(nkt):
                        j = kt - 4 * qm     # >=0 on diagonal blocks
                        o0 = max(j, 0) * 128
                        sw = sw01[kt % 2]
                        for hh in range(2):
                            nc.tensor.matmul(
                                sw[:, hh * 512 + o0:(hh + 1) * 512],
                                kTh[hh * 64:(hh + 1) * 64,
                                    kt * 128:(kt + 1) * 128],
                                qTh[hh * 64:(hh + 1) * 64,
                                    qm * 512 + o0:(qm + 1) * 512],
                                start=True, stop=True)
                        pt = pts.tile([128, 1024], BF16, tag="pT", name="pT")
                        swv = sw[:].rearrange("p (a q) -> p a q", a=2)
                        ptv = pt[:].rearrange("p (a q) -> p a q", a=2)
                        nc.scalar.activation(
                            ptv[:, :, o0:512], swv[:, :, o0:512],
                            mybir.ActivationFunctionType.Exp, scale=SCALE)
                        if j >= 0:      # diagonal block: 0/1 mask both heads
                            mv = mask_sb[:, j, :].rearrange(
                                "p (a q) -> p a q", a=2)
                            nc.vector.tensor_mul(
                                ptv[:, :, o0:512], ptv[:, :, o0:512],
                                mv[:, :, o0:512])
                        for hh in range(2):
                            h = 2 * p + hh
                            nc.tensor.matmul(
                                oa[hh][:, o0:512],
                                vt[kt][:, h, :],
                                pt[:, hh * 512 + o0:(hh + 1) * 512],
                                start=(kt == 0), stop=(kt == nkt - 1),
                                skip_group_check=True)
                        emit_filler(2)
                    for hh in range(2):
                        oacc = oa[hh]
                        den = sml.tile([1, 512], F32, tag="den", name="den")
                        nc.vector.tensor_copy(den[:], oacc[HD:HD + 1, 0:512])
                        rd = sml.tile([1, 512], F32, tag="rd", name="rd")
                        nc.vector.reciprocal_approx_fast(rd[:], den[:])
                        bcs = sml.tile([HD, 512], F32, tag="bcs", name="bcs")
                        nc.gpsimd.partition_broadcast(bcs[:], rd[:])
                        nc.vector.tensor_mul(
                            outT[p][hh * 64:(hh + 1) * 64,
                                    qm * 512:(qm + 1) * 512],
                            oacc[0:HD, 0:512], bcs[:])
                    emit_filler(4)
                drain_filler()

            # ---- P(3): tail projection for the last quarter ----
            queue_p_phase(3)
            drain_filler()

    nc.compile()
    return nc


def _make_masks():
    k = np.arange(128)[:, None, None]
    j = np.arange(4)[None, :, None]
    q = np.arange(512)[None, None, :]
    m = (j * 128 + k <= q)                       # [128, 4, 512]
    m2 = np.concatenate([m, m], axis=2)          # [128, 4, 1024] (both heads)
    return m2.astype(ml_dtypes.bfloat16)


def make_in_maps(x, w_qkv, w_proj):
    bf = ml_dtypes.bfloat16
    masks = _make_masks()
    in_maps = []
    for c in range(N_CORES):
        b, g = c // 2, c % 2
        # [p, kc, t]: partition p = channel-within-chunk, kc = chunk
        xTv = np.ascontiguousarray(
            x[b].T.reshape(KC, 128, T).transpose(1, 0, 2).astype(bf))
        wq_ = w_qkv[:, g * CH:(g + 1) * CH]
        wk_ = w_qkv[:, C + g * CH:C + (g + 1) * CH]
        stacked = np.concatenate([wq_, wk_], axis=1)         # [1024, 1024]
        # [p, ft, kc, f]
        w_qk = np.ascontiguousarray(
            stacked.reshape(KC, 128, FT, 128).transpose(1, 2, 0, 3).astype(bf))
        # [p, kc, ch]
        w_v = np.ascontiguousarray(
            w_qkv[:, 2 * C + g * CH:2 * C + (g + 1) * CH]
            .reshape(KC, 128, CH).transpose(1, 0, 2).astype(bf))
        # [p, ct, c]
        w_pj = np.ascontiguousarray(
            w_proj[g * CH:(g + 1) * CH, :]
            .reshape(CT, 128, C).transpose(1, 0, 2).astype(bf))
        in_maps.append({
            "xT": xTv, "w_qk": w_qk, "w_v": w_v, "w_pj": w_pj,
            "masks": masks,
        })
    return in_maps


def kernel(x, w_qkv, w_proj):
    x = np.asarray(x, dtype=np.float32)
    w_qkv = np.asarray(w_qkv, dtype=np.float32)
    w_proj = np.asarray(w_proj, dtype=np.float32)

    if "nc" not in _CACHE:
        _CACHE["nc"] = build_kernel()
    nc = _CACHE["nc"]

    in_maps = make_in_maps(x, w_qkv, w_proj)
    res = run_bass_kernel_spmd(nc, in_maps, core_ids=list(range(N_CORES)))
    _CACHE["last_result"] = res

    yout = np.empty((B, T, C), dtype=np.float32)
    for b in range(B):
        yout[b] = res.results[2 * b]["y"] + res.results[2 * b + 1]["y"]
    return yout
